# revision 1
# baseline (speedup 1.0000x reference)
"""Trainium2 Bass kernel for the Sobel/gabor depthwise-conv + elementwise chain.

reference:
    gx = depthwise3x3(x, KX); gy = depthwise3x3(x, KY)       # SAME zero-pad
    d  = x + 0.001
    gabor = arctan(sqrt((gx/d)^2 + (gy/d)^2)) / 255
    gabor = (gabor - MEAN[c]) / STD[c]
    return (gabor, x)

Kernel strategy (pure data parallel, batch 32 -> 8 cores x 4 images):
  * Both 3x3 kernels are separable: KX = a (x) b, KY = c (x) a with
    a=[s,1,s], b=[-1,0,1], c=[1,0,-1], s=1/(2*sqrt(2)).
  * The vertical (partition-dim) conv runs on TensorE as banded-matrix
    matmuls; the horizontal +-1 shifts are folded into the SAME matmuls by
    slicing the moving operand / PSUM output along the free dim and
    accumulating in PSUM.  gx needs 2 taps (+A @ w+1, -A @ w-1), gy needs 3
    (C @ w, sC @ w-1, sC @ w+1).
  * H=512 rows are covered by 5 row-tiles of <=128 input rows producing
    127/126/126/126/7 output rows (input tiles overlap by 2 rows), with
    top/interior/bottom band variants encoding the zero padding.
  * Elementwise chain uses atan(sqrt(t)/d) = pi/2 - atan(d * rsqrt(t)):
      sq   = square(gx|gy)              ACT or DVE (alternating, balance)
      t    = sqx + sqy                  DVE  (bf16)
      w    = Abs_reciprocal_sqrt(t+eps) ACT
      v    = (x+0.001) * w              DVE  (fp16 * bf16)
      g    = Arctan(v)                  ACT
      out  = g * k1 + k2               DVE  tensor_scalar dual-op -> f32
    fp16 for the conv input (precision: 4.1e-4 scale-rel absmax vs f32 ref),
    bf16 for chain intermediates (wide exponent avoids rsqrt overflow).
  * ACT table sets: Square+Abs_reciprocal_sqrt live in
    abs_reciprocal_sqrt_and_small, Arctan in sigmoid_and_others.  ACT ops are
    chained in emission order and emitted in two phases per 6-group block so
    only ~4 table switches occur.
"""

import numpy as np
from contextlib import ExitStack

N_FULL, C, H, W = 32, 3, 512, 512
N_CORES = 8
NPC = N_FULL // N_CORES          # images per core
GROUPS_FULL = NPC * C            # (n, c) groups per core

S = 1.0 / (2.0 * np.sqrt(2.0))
MEAN = (0.485, 0.456, 0.406)
STD = (0.229, 0.224, 0.225)

# 5 row-tiles covering H=512: input rows [r0, r0+K).  Band matrices map
# PSUM/output partition m <-> global row r0+m (partition-aligned with the
# input tile), with out-of-tile columns zeroed; stores skip invalid partitions.
R0 = (0, 126, 252, 378, 504)
KJ = (128, 128, 128, 128, 8)     # input rows per tile
MOPS = (127, 127, 127, 127, 8)   # partitions carried through the chain
SOFF = (0, 1, 1, 1, 1)           # first valid partition at store time
SM = (127, 126, 126, 126, 7)     # valid output rows per tile
VAR = (1, 0, 0, 0, 2)            # 0=interior 1=top 2=bottom band variant

PHASE_GROUPS = 6                 # groups per ACT table-set phase
RSQRT_BIAS = 1e-24               # AbsRsqrt valid range floor is ~2^-87


def make_bands() -> np.ndarray:
    """[128, 12*128] fp16 stationary matrices. Column block (var*4+s)*128 holds
    band variant var for coeff set s in {A, -A, C, sC}.  Column m produces
    output row r0+m from input rows k=m-1..m+1 (B[k,m] = w[k-m+1]); columns
    whose output row lies outside the tile's valid range are zeroed (top
    variant keeps m=0 with the k=-1 tap dropped = zero padding)."""
    a = np.array([S, 1.0, S], np.float32)
    c = np.array([1.0, 0.0, -1.0], np.float32)
    sets = [a, -a, c, S * c]
    # block 12 stays all-zero: used as the start=True matmul that zero-fills
    # the gx PSUM bank (HW zero-region semantics allow only ONE start per bank)
    bands = np.zeros((128, 13 * 128), np.float32)
    for var in range(3):
        kmax = 7 if var == 2 else 127          # last valid input row index
        mlo = 0 if var == 1 else 1             # col 0 zeroed unless top
        mhi = 7 if var == 2 else 126
        for si, wv in enumerate(sets):
            blk = bands[:, (var * 4 + si) * 128:(var * 4 + si) * 128 + 128]
            for m in range(mlo, mhi + 1):
                for d in range(3):
                    k = m + d - 1
                    if 0 <= k <= kmax:
                        blk[k, m] = wv[d]
    return bands.astype(np.float16)


def build_nc(groups: int = GROUPS_FULL, sq_mode: str = "alt"):
    """Build + compile the per-core Bass program.

    DRAM I/O: x [groups*512, 512] f32, bands [128, 1536] f16,
              gabor [groups*512, 512] f32.
    """
    from concourse import bacc, mybir, tile
    import concourse.bass as bass

    f32 = mybir.dt.float32
    f16 = mybir.dt.float16
    bf16 = mybir.dt.bfloat16
    AF = mybir.ActivationFunctionType
    ALU = mybir.AluOpType

    nc = bacc.Bacc("TRN2", target_bir_lowering=False, debug=False)
    x_d = nc.declare_dram_parameter("x", [groups * H, W], f32, isOutput=False)
    b_d = nc.declare_dram_parameter("bands", [128, 13 * 128], f16, isOutput=False)
    o_d = nc.declare_dram_parameter("gabor", [groups * H, W], f32, isOutput=True)

    act_prev = [None]

    def chain(bi):
        # serialize ACT in emission order so table-set phasing holds
        if act_prev[0] is not None:
            bass._add_dep_helper(bi.ins, act_prev[0].ins, sync=False,
                                 reason="ACT table-set order")
        act_prev[0] = bi
        return bi

    WG = 5 * W  # per-group wide free dim (5 row-tiles side by side)

    with tile.TileContext(nc) as tc, ExitStack() as ctx:
        cpool = ctx.enter_context(tc.tile_pool(name="const", bufs=1))
        xpool = ctx.enter_context(tc.tile_pool(name="xraw", bufs=2))
        hpool = ctx.enter_context(tc.tile_pool(name="xh", bufs=3))
        qpool = ctx.enter_context(tc.tile_pool(name="sq", bufs=2))
        tpool = ctx.enter_context(tc.tile_pool(name="t", bufs=2))
        wpool = ctx.enter_context(tc.tile_pool(name="w", bufs=2))
        vpool = ctx.enter_context(tc.tile_pool(name="v", bufs=PHASE_GROUPS + 2))
        gpool = ctx.enter_context(tc.tile_pool(name="g", bufs=2))
        opool = ctx.enter_context(tc.tile_pool(name="o", bufs=2))
        ppool = ctx.enter_context(tc.tile_pool(name="psum", bufs=4, space="PSUM"))

        bands_sb = cpool.tile([128, 13 * 128], f16)
        nc.sync.dma_start(out=bands_sb[:], in_=b_d[:, :])
        bias_t = cpool.tile([128, 1], f32)
        nc.vector.memset(bias_t[:], RSQRT_BIAS)

        def band(var, si, K):
            # full 128 columns: invalid output rows get zero coefficients, so
            # every PSUM partition is written (downstream ops read [0:128])
            off = (var * 4 + si) * 128
            return bands_sb[0:K, off:off + 128]

        for p0 in range(0, groups, PHASE_GROUPS):
            pend = min(p0 + PHASE_GROUPS, groups)
            vtiles = {}
            # ---- phase A: conv + square + t + rsqrt + v  (abs_rsqrt set) ----
            for g in range(p0, pend):
                x_raw = xpool.tile([128, WG], f32)
                for j in range(5):
                    row = g * H + R0[j]
                    nc.sync.dma_start(out=x_raw[0:KJ[j], j * W:(j + 1) * W],
                                      in_=x_d[row:row + KJ[j], :])
                # fill chunk-4's unused partitions with (any) valid data so the
                # full-width elementwise ops never see uninitialized memory
                nc.sync.dma_start(out=x_raw[8:128, 4 * W:5 * W],
                                  in_=x_raw[8:128, 0:W])
                xh = hpool.tile([128, WG], f16)
                nc.vector.tensor_scalar_add(xh[:, :], x_raw[:, :], 0.001)

                sq = qpool.tile([128, 2 * WG], bf16)
                for j in range(5):
                    K, var = KJ[j], VAR[j]
                    xj = xh[0:K, j * W:(j + 1) * W]
                    ps = ppool.tile([128, 1024], f32)
                    gx = ps[:, 0:512]
                    gy = ps[:, 512:1024]
                    mm = nc.tensor.matmul
                    # gx = A @ x[w+1] - A @ x[w-1]
                    # ONE start=True per PSUM bank (zero-weight K=1 zero-fill),
                    # then accumulate: HW start semantics are zero-region wide.
                    mm(gx[:, 0:512], bands_sb[0:1, 12 * 128:12 * 128 + 128],
                       xj[0:1, 0:512], start=True, stop=False,
                       skip_group_check=True)
                    mm(gx[:, 0:511], band(var, 0, K), xj[:, 1:512],
                       start=False, stop=False, skip_group_check=True)
                    mm(gx[:, 1:512], band(var, 1, K), xj[:, 0:511],
                       start=False, stop=True, skip_group_check=True)
                    # gy = C @ x[w] + sC @ x[w-1] + sC @ x[w+1]
                    mm(gy[:, 0:512], band(var, 2, K), xj[:, 0:512],
                       start=True, stop=False, skip_group_check=True)
                    mm(gy[:, 1:512], band(var, 3, K), xj[:, 0:511],
                       start=False, stop=False, skip_group_check=True)
                    mm(gy[:, 0:511], band(var, 3, K), xj[:, 1:512],
                       start=False, stop=True, skip_group_check=True)
                    chain(nc.scalar.activation(
                        sq[:, j * 1024:(j + 1) * 1024], ps[:, :], AF.Square))

                # t[j*512+w] = sq[j*1024+w] + sq[j*1024+512+w] for all 5 j
                tt = tpool.tile([128, WG], bf16)
                sq3 = sq[:].rearrange("p (j two w) -> p j two w", two=2, w=W)
                nc.vector.tensor_add(
                    tt[:].rearrange("p (j w) -> p j w", w=W),
                    sq3[:, :, 0, :], sq3[:, :, 1, :])

                wt = wpool.tile([128, WG], bf16)
                chain(nc.scalar.activation(wt[:, :], tt[:, :],
                                           AF.Abs_reciprocal_sqrt,
                                           bias=bias_t[:, 0:1]))

                v = vpool.tile([128, WG], bf16)
                nc.vector.tensor_mul(v[:, :], xh[:, :], wt[:, :])
                vtiles[g] = v

            # ---- phase B: arctan + affine + store  (sigmoid set) ----
            for g in range(p0, pend):
                cch = g % C
                k1 = float(-1.0 / (255.0 * STD[cch]))
                k2 = float((np.pi / 2.0 / 255.0 - MEAN[cch]) / STD[cch])
                v = vtiles.pop(g)
                # f32 arctan output: the affine below computes at input dtype,
                # so a bf16 ga would round (ga*k1)+k2 to bf16 (~8e-3 abs err)
                ga = gpool.tile([128, WG], f32)
                chain(nc.scalar.activation(ga[:, :], v[:, :], AF.Arctan))
                ot = opool.tile([128, WG], f32)
                nc.vector.tensor_scalar(ot[:, :], ga[:, :], k1, k2,
                                        ALU.mult, ALU.add)
                for j in range(5):
                    soff, sm = SOFF[j], SM[j]
                    row = g * H + R0[j] + soff
                    nc.sync.dma_start(
                        out=o_d[row:row + sm, :],
                        in_=ot[soff:soff + sm, j * W:(j + 1) * W])

    nc.compile()
    return nc


_NC_CACHE = {}


def _get_nc(groups=GROUPS_FULL, sq_mode="alt"):
    key = (groups, sq_mode)
    if key not in _NC_CACHE:
        _NC_CACHE[key] = build_nc(groups, sq_mode)
    return _NC_CACHE[key]


def run(x: np.ndarray, trace: bool = False, **spmd_kwargs):
    """x: [32,3,512,512] f32 -> gabor [32,3,512,512] f32 (device part only)."""
    from concourse.bass_utils import run_bass_kernel_spmd

    x = np.ascontiguousarray(np.asarray(x, dtype=np.float32))
    assert x.shape == (N_FULL, C, H, W), x.shape
    nc = _get_nc()
    bands = make_bands()
    shards = [
        np.ascontiguousarray(
            x[i * NPC:(i + 1) * NPC].reshape(GROUPS_FULL * H, W))
        for i in range(N_CORES)
    ]
    in_maps = [{"x": s, "bands": bands} for s in shards]
    res = run_bass_kernel_spmd(nc, in_maps, list(range(N_CORES)),
                               trace=trace, **spmd_kwargs)
    outs = [
        np.asarray(res.results[i]["gabor"], np.float32)
        .reshape(NPC, C, H, W)
        for i in range(N_CORES)
    ]
    gabor = np.concatenate(outs, axis=0)
    return gabor, res


def kernel(x: np.ndarray):
    xin = np.asarray(x)
    gabor, _ = run(xin)
    return (gabor, xin.astype(np.float32, copy=False))

